# revision 18
# baseline (speedup 1.0000x reference)
"""Chamfer distance (L2) Bass kernel for 8 TRN2 NeuronCores.

Problem: xyz1 [B=8, N=8192, 3] f32, xyz2 [B=8, M=8192, 3] f32.
  d[b, n, m] = |xyz1[b,n] - xyz2[b,m]|^2
  dist1[b, n] = min_m d[b, n, m];  dist2[b, m] = min_n d[b, n, m]

Sharding: data-parallel over batch — core b handles batch b (B == n_cores == 8).
No collectives; outputs are gathered on the host.

Per-core algorithm — single pass over the distance matrix:
  d = x2[n] + y2[m] - 2*x.y is produced tile-by-tile by ONE TensorE matmul per
  output tile via an augmented K=13 bf16 contraction: hi/lo splits of the
  coordinates give ~fp16-accurate cross terms at full bf16 PE rate, and
  ones-rows fold the squared-norm offsets in, so fp32 PSUM tiles hold finished
  d values. Each PSUM group [128, 2048] is then:
    1. copied to an fp16 SBUF staging tile by ScalarE (the only other engine
       that can read PSUM) — this moves the expensive 1x PSUM read off the DVE;
    2. min-accumulated elementwise into acc2[128, M] fp16 by a DVE
       tensor_tensor in the fast all-16-bit 2x mode (column mins -> dist2);
    3. row-reduced by a DVE tensor_tensor_reduce over the two halves of the
       staging tile: out(scratch) = min(left, right) and
       rowp[:, i] = min(reduce_min(out), rowp[:, i]) — both inputs belong to
       the same 128 output rows, so the fused reduce is exact (dist1).
  dist1 comes straight from rowp; dist2's final min over the 128 partition
  lanes of acc2 uses PE transposes of 128x128 blocks + free-axis reduce_min.
"""

import sys

if "/opt/trn_rl_repo" not in sys.path:
    sys.path.insert(0, "/opt/trn_rl_repo")

import numpy as np
import ml_dtypes

import concourse.bass as bass  # noqa: F401
import concourse.mybir as mybir
import concourse.tile as tile
from concourse import bacc
from concourse.bass_utils import run_bass_kernel_spmd
from concourse.masks import make_identity

BF16 = ml_dtypes.bfloat16

B = 8
N = 8192
M = 8192
P = 128  # output rows per tile (partition dim)
K = 13  # augmented contraction rows
BIG = 60000.0  # min-identity; finite in fp16, >> any squared distance here
ST = "float16"  # staging/accumulator dtype: 16-bit for DVE 2x mode, 2^-11 rounding

_NC_CACHE = {}


def _emit_transposed(tc, nc, pool, vec_sb, ident, out_dram):
    """vec_sb [P, n_blk] fp16 holds out[i*P + p] at [p, i]. PE-transpose to
    [n_blk, P], cast-copy to fp32, and DMA out contiguously (the direct
    [p, i]-strided DMA would scatter 4-byte elements)."""
    n_blk = vec_sb.shape[1]
    st = getattr(mybir.dt, ST)
    with tc.tile_pool(name="psum_o", bufs=1, space="PSUM") as psum_o:
        pt = psum_o.tile([n_blk, P], st)
        nc.tensor.transpose(pt[:, :], vec_sb[:, :], ident[:, :])
        ot = pool.tile([n_blk, P], mybir.dt.float32, tag="out_t")
        nc.vector.tensor_copy(ot[:, :], pt[:, :])
        nc.sync.dma_start(
            out=out_dram.ap().rearrange("(i p) -> i p", p=P), in_=ot[:, :]
        )


def _part_min_out(tc, nc, pool, acc, ident, out_dram):
    """Min over the 128 partitions of acc -> [m] via PE transpose + reduce."""
    m_len = acc.shape[1]
    n_blk = m_len // P
    st = getattr(mybir.dt, ST)
    osb = pool.tile([P, n_blk], st, tag="partmin_out")
    with tc.tile_pool(name="psum_t", bufs=2, space="PSUM") as psum_t:
        for t in range(n_blk):
            pst = psum_t.tile([P, P], st)
            nc.tensor.transpose(pst[:, :], acc[:, t * P : (t + 1) * P], ident[:, :])
            nc.vector.tensor_reduce(
                out=osb[:, t : t + 1],
                in_=pst[:, :],
                axis=mybir.AxisListType.X,
                op=mybir.AluOpType.min,
            )
    _emit_transposed(tc, nc, pool, osb, ident, out_dram)


def build_nc(n, m, mm_free=512, ps_group=2048, reps=1, gps_mod=0, act_mod=0):
    """Build + compile the per-core Bass program (SPMD, same on all cores).

    reps>1 repeats the main pass (identical results — min is idempotent);
    used only for timing: kernel time = slope of wall time vs reps.
    """
    ps_group = min(ps_group, m)
    mm_free = min(mm_free, ps_group)
    half = ps_group // 2
    st = getattr(mybir.dt, ST)
    n_tiles = n // P
    n_groups = m // ps_group

    nc = bacc.Bacc("TRN2", target_bir_lowering=False, debug=False)
    sx = nc.dram_tensor("sx", [K, n], mybir.dt.bfloat16, kind="ExternalInput")
    my = nc.dram_tensor("my", [K, m], mybir.dt.bfloat16, kind="ExternalInput")
    d1 = nc.dram_tensor("dist1", [n], mybir.dt.float32, kind="ExternalOutput")
    d2 = nc.dram_tensor("dist2", [m], mybir.dt.float32, kind="ExternalOutput")

    with tile.TileContext(nc) as tc:
        with tc.tile_pool(name="singles", bufs=1) as singles:
            sx_sb = singles.tile([K, n], mybir.dt.bfloat16)
            my_sb = singles.tile([K, m], mybir.dt.bfloat16)
            nc.sync.dma_start(out=sx_sb[:, :], in_=sx.ap())
            nc.sync.dma_start(out=my_sb[:, :], in_=my.ap())

            acc2 = singles.tile([P, m], st)
            rowp = singles.tile([P, n_tiles], st)
            nc.vector.memset(acc2[:, :], BIG)

            with (
                tc.tile_pool(name="psum", bufs=2, space="PSUM") as psum_pool,
                tc.tile_pool(name="cp", bufs=3) as cp_pool,
                tc.tile_pool(name="scr", bufs=3) as scr_pool,
                tc.tile_pool(name="rowacc", bufs=2) as rowacc_pool,
            ):
                import contextlib

                rep_ctx = (
                    tc.For_i(0, reps, 1) if reps > 1 else contextlib.nullcontext()
                )
                with rep_ctx:
                  for i in range(n_tiles):
                    lhsT = sx_sb[:, i * P : (i + 1) * P]
                    # dist1-chain engine: offload some row-tiles to GPSIMD
                    d1eng = (
                        nc.gpsimd
                        if (gps_mod > 0 and i % gps_mod == 0)
                        else nc.vector
                    )
                    rowacc = rowacc_pool.tile([P, half], st, tag="rowacc")
                    for g in range(n_groups):
                        ps = psum_pool.tile([P, ps_group], mybir.dt.float32)
                        for t in range(ps_group // mm_free):
                            lo = g * ps_group + t * mm_free
                            nc.tensor.matmul(
                                ps[:, t * mm_free : (t + 1) * mm_free],
                                lhsT=lhsT,
                                rhs=my_sb[:, lo : lo + mm_free],
                                start=True,
                                stop=True,
                            )
                        sl = acc2[:, g * ps_group : (g + 1) * ps_group]
                        direct = act_mod > 0 and (i * n_groups + g) % act_mod == 0
                        if direct:
                            # DVE consumes PSUM without ScalarE staging:
                            # relieves ACT at the cost of 1x-mode DVE reads.
                            nc.vector.tensor_tensor(
                                out=sl, in0=ps[:, :], in1=sl, op=mybir.AluOpType.min
                            )
                            t1 = scr_pool.tile([P, half], st, tag="scr")
                            nc.vector.tensor_tensor(
                                out=t1[:, :],
                                in0=ps[:, :half],
                                in1=ps[:, half:],
                                op=mybir.AluOpType.min,
                            )
                            if g == 0:
                                nc.vector.tensor_copy(rowacc[:, :], t1[:, :])
                            else:
                                nc.vector.tensor_tensor(
                                    out=rowacc[:, :],
                                    in0=t1[:, :],
                                    in1=rowacc[:, :],
                                    op=mybir.AluOpType.min,
                                )
                            continue
                        cp = cp_pool.tile([P, ps_group], st, tag="cp")
                        nc.scalar.copy(out=cp[:, :], in_=ps[:, :])
                        # dist2 partial: acc2 = min(acc2, d)
                        nc.vector.tensor_tensor(
                            out=sl, in0=cp[:, :], in1=sl, op=mybir.AluOpType.min
                        )
                        # dist1 partial: rowacc = min over groups of
                        # min(cp_left, cp_right)
                        if g == 0:
                            d1eng.tensor_tensor(
                                out=rowacc[:, :],
                                in0=cp[:, :half],
                                in1=cp[:, half:],
                                op=mybir.AluOpType.min,
                            )
                        else:
                            t1 = scr_pool.tile([P, half], st, tag="scr")
                            d1eng.tensor_tensor(
                                out=t1[:, :],
                                in0=cp[:, :half],
                                in1=cp[:, half:],
                                op=mybir.AluOpType.min,
                            )
                            d1eng.tensor_tensor(
                                out=rowacc[:, :],
                                in0=t1[:, :],
                                in1=rowacc[:, :],
                                op=mybir.AluOpType.min,
                            )
                    nc.vector.tensor_reduce(
                        out=rowp[:, i : i + 1],
                        in_=rowacc[:, :],
                        axis=mybir.AxisListType.X,
                        op=mybir.AluOpType.min,
                    )

            ident = singles.tile([P, P], st)
            make_identity(nc, ident[:, :])
            _emit_transposed(tc, nc, singles, rowp, ident, d1)
            _part_min_out(tc, nc, singles, acc2, ident, d2)

    nc.compile()
    return nc


def get_nc(n=N, m=M, reps=1, gps_mod=0, act_mod=0):
    key = (n, m, reps, gps_mod, act_mod)
    if key not in _NC_CACHE:
        _NC_CACHE[key] = build_nc(n, m, reps=reps, gps_mod=gps_mod, act_mod=act_mod)
    return _NC_CACHE[key]


def _split_hi_lo(a):
    hi = a.astype(BF16)
    lo = (a - hi.astype(np.float32)).astype(BF16)
    return hi, lo


def _stat_rows(u, u2h, u2l):
    """Stationary-side augmented rows [13, len] for points u [len, 3] f32."""
    uh, ul = _split_hi_lo(u)
    out = np.empty((K, u.shape[0]), BF16)
    out[0:3] = uh.T
    out[3:6] = uh.T
    out[6:9] = ul.T
    out[9] = BF16(1.0)
    out[10] = BF16(1.0)
    out[11] = u2h
    out[12] = u2l
    return np.ascontiguousarray(out)


def _mov_rows(v, v2h, v2l):
    """Moving-side augmented rows [13, len] for points v [len, 3] f32."""
    vh, vl = _split_hi_lo(v)
    out = np.empty((K, v.shape[0]), BF16)
    out[0:3] = (-2.0 * vh.astype(np.float32)).astype(BF16).T
    out[3:6] = (-2.0 * vl.astype(np.float32)).astype(BF16).T
    out[6:9] = out[0:3]
    out[9] = v2h
    out[10] = v2l
    out[11] = BF16(1.0)
    out[12] = BF16(1.0)
    return np.ascontiguousarray(out)


def _prep_core_inputs(x, y):
    """Augmented bf16 matrices for one batch: core computes d[n-tile, m] tiles
    with x stationary and y moving; both reductions happen in the same pass."""
    x = np.asarray(x, np.float32)
    y = np.asarray(y, np.float32)
    x2 = np.sum(x.astype(np.float64) * x, axis=-1).astype(np.float32)
    y2 = np.sum(y.astype(np.float64) * y, axis=-1).astype(np.float32)
    x2h, x2l = _split_hi_lo(x2)
    y2h, y2l = _split_hi_lo(y2)
    return {
        "sx": _stat_rows(x, x2h, x2l),
        "my": _mov_rows(y, y2h, y2l),
    }


def kernel(xyz1, xyz2):
    xyz1 = np.asarray(xyz1, np.float32)
    xyz2 = np.asarray(xyz2, np.float32)
    b, n, _ = xyz1.shape
    m = xyz2.shape[1]
    assert b == B and n == N and m == M, (b, n, m)

    nc = get_nc(n, m)
    in_maps = [_prep_core_inputs(xyz1[i], xyz2[i]) for i in range(b)]
    res = run_bass_kernel_spmd(nc, in_maps, core_ids=list(range(b)))
    dist1 = np.stack([res.results[i]["dist1"] for i in range(b)]).astype(np.float32)
    dist2 = np.stack([res.results[i]["dist2"] for i in range(b)]).astype(np.float32)
    return dist1, dist2
